# revision 1
# baseline (speedup 1.0000x reference)
"""Distributed Trainium2 kernel for the dense transformer block.

Strategy (8 NeuronCores, SPMD):
  Phase A (token-parallel): each core owns 512 contiguous tokens (+3-token
    causal-conv halo). rmsnorm -> qkv matmul -> depthwise causal conv ->
    SiLU -> RoPE, all in feature-major layout (channels on partitions).
  AllToAll 1: reshard q/k/v from token-parallel to head-parallel.
  Phase B (head-parallel): each core runs causal flash-attention (no
    running max; scores are tiny for this problem) for its 2 heads over
    all 4096 tokens.
  AllToAll 2: reshard attention output y back to token-parallel.
  Phase C (token-parallel): proj + residual -> rmsnorm2 -> gated MLP ->
    residual. Output is feature-major (2048, 512) per core; the host
    reassembles (B, T, C).

All matmuls run with bf16 operands and f32 PSUM accumulation. Norm
scales, conv accumulation, residuals and softmax denominators stay f32.
"""
import os
import sys

sys.path.insert(0, "/opt/trn_rl_repo")

import numpy as np
import ml_dtypes

import concourse.bass as bass
import concourse.mybir as mybir
from concourse import bacc, tile
from concourse.bass_utils import run_bass_kernel_spmd

B, T, C = 2, 2048, 2048
NH, NG, HS = 16, 4, 128
QPK = NH // NG
DCONV = 4
IM = 5632
EPS = 1e-5
NCORES = 8
TOK = 512            # tokens per core
HALO = DCONV - 1
XW = TOK + HALO      # 515
CH = 259             # chunk width with halo (256 + 3)
NKC = C // 128       # 16
NMQ = (NH + 2 * NG)  # 24 qkv m-tiles
NMI = IM // 128      # 44
SCALE = 1.0 / float(np.sqrt(HS))

F32 = mybir.dt.float32
BF16 = mybir.dt.bfloat16
AF = mybir.ActivationFunctionType
ALU = mybir.AluOpType

DEBUG = bool(int(os.environ.get("KERNEL_DEBUG", "0")))
TRACE = bool(int(os.environ.get("KERNEL_TRACE", "0")))

LAST_RESULTS = None  # test.py reads exec_time from here


# --------------------------------------------------------------------------
# builder
# --------------------------------------------------------------------------

def build_nc():
    nc = bacc.Bacc("TRN2", target_bir_lowering=False, debug=False,
                   enable_asserts=True, num_devices=NCORES)

    x_d = nc.dram_tensor("x", [C, XW], F32, kind="ExternalInput")
    wq_d = nc.dram_tensor("wq", [NMQ, 128, C], BF16, kind="ExternalInput")
    wp_d = nc.dram_tensor("wp", [16, 128, C], BF16, kind="ExternalInput")
    w1_d = nc.dram_tensor("w1", [NMI, 128, C], BF16, kind="ExternalInput")
    w2_d = nc.dram_tensor("w2", [NMI, 128, C], BF16, kind="ExternalInput")
    wm_d = nc.dram_tensor("wm", [16, 128, IM], BF16, kind="ExternalInput")
    cw_d = nc.dram_tensor("cw", [128, NMQ * DCONV], F32, kind="ExternalInput")
    trig_d = nc.dram_tensor("trig", [128, 1024], BF16, kind="ExternalInput")
    msk_d = nc.dram_tensor("msk", [128, 2048], BF16, kind="ExternalInput")
    sel_d = nc.dram_tensor("sel", [8, 1024], BF16, kind="ExternalInput")
    rotm_d = nc.dram_tensor("rotm", [128, 128], BF16, kind="ExternalInput")
    out_d = nc.dram_tensor("out", [C, TOK], F32, kind="ExternalOutput")

    dbg = {}
    if DEBUG:
        dbg["sl"] = nc.dram_tensor("d_sl", [NMQ * 128, TOK], BF16, kind="ExternalOutput")
        dbg["t1o"] = nc.dram_tensor("d_t1o", [4096, 512], BF16, kind="ExternalOutput")
        dbg["y"] = nc.dram_tensor("d_y", [256, B * T], BF16, kind="ExternalOutput")
        dbg["x2"] = nc.dram_tensor("d_x2", [C, TOK], F32, kind="ExternalOutput")

    with tile.TileContext(nc) as tc:
        with tc.tile_pool(name="dram", bufs=1, space="DRAM") as dram, \
             tc.tile_pool(name="pers", bufs=1) as pers:
            t1i_kv = dram.tile([2048, 512], BF16)
            t1o_kv = dram.tile([2048, 512], BF16)
            t1i_q = dram.tile([2048, 512], BF16)
            t1o_q = dram.tile([2048, 512], BF16)
            t2i_a = dram.tile([1024, 512], BF16)
            t2o_a = dram.tile([1024, 512], BF16)
            t2i_b = dram.tile([1024, 512], BF16)
            t2o_b = dram.tile([1024, 512], BF16)

            # ---- constants ----
            cw_sb = pers.tile([128, NMQ * DCONV], F32, tag="cw", name="cw")
            trig_sb = pers.tile([128, 1024], BF16, tag="trig", name="trig")
            msk_sb = pers.tile([128, 2048], BF16, tag="msk", name="msk")
            sel_sb = pers.tile([8, 1024], BF16, tag="sel", name="sel")
            rotm = pers.tile([128, 128], BF16, tag="rotm", name="rotm")
            nc.sync.dma_start(cw_sb[:], cw_d[:])
            nc.sync.dma_start(trig_sb[:], trig_d[:])
            nc.sync.dma_start(msk_sb[:], msk_d[:])
            nc.sync.dma_start(sel_sb[:], sel_d[:])
            nc.sync.dma_start(rotm[:], rotm_d[:])

            ones128 = pers.tile([128, 128], BF16, tag="ones128", name="ones128")
            eps1 = pers.tile([1, 1], F32, tag="eps1", name="eps1")
            nc.gpsimd.memset(ones128[:], 1.0)
            nc.gpsimd.memset(eps1[:], EPS)

            # ---- persistent activations ----
            xh = [pers.tile([128, XW], F32, tag=f"xh{i}", name=f"xh{i}") for i in range(NKC)]
            yk = [pers.tile([128, TOK], BF16, tag=f"yk{i}", name=f"yk{i}")
                  for i in range(NKC)]
            for i in range(NKC):
                nc.sync.dma_start(xh[i][:], x_d[i * 128:(i + 1) * 128, :])

            # ============================================================
            # Phase A: norm1 -> qkv -> conv -> silu -> rope -> pack A2A1
            # ============================================================
            with tc.tile_pool(name="pa_sb", bufs=1) as pa, \
                 tc.tile_pool(name="pa_ps", bufs=1, space="PSUM") as pap:
                n1 = [pa.tile([128, 2, CH], BF16, tag=f"n1_{i}", name=f"n1_{i}")
                      for i in range(NKC)]
                for ch in range(2):
                    ss_ps = pap.tile([128, CH], F32, tag="ps1", bufs=3, name="ps1")
                    for kk in range(NKC):
                        xsq = pa.tile([128, CH], BF16, tag="xsq", bufs=3, name="xsq")
                        nc.scalar.activation(xsq[:], xh[kk][:, ch * 256:ch * 256 + CH],
                                             AF.Square)
                        nc.tensor.matmul(ss_ps[:], ones128[:], xsq[:],
                                         start=(kk == 0), stop=(kk == NKC - 1))
                    rt = pa.tile([1, CH], F32, tag="rt", bufs=2, name="rt")
                    nc.scalar.activation(rt[:], ss_ps[0:1, :], AF.Sqrt,
                                         bias=eps1[:], scale=1.0 / C)
                    rinv = pa.tile([1, CH], F32, tag="rinv", bufs=2, name="rinv")
                    nc.vector.reciprocal(rinv[:], rt[:])
                    rb_sb = pa.tile([128, CH], F32, tag="rb", bufs=2, name="rb")
                    nc.gpsimd.partition_broadcast(rb_sb[:], rinv[:])
                    for kk in range(NKC):
                        nc.vector.tensor_mul(n1[kk][:, ch, :],
                                             xh[kk][:, ch * 256:ch * 256 + CH],
                                             rb_sb[:])

                m_order = [g * 6 + sl for g in range(NG) for sl in (4, 5)] + \
                          [g * 6 + sl for g in range(NG) for sl in range(4)]
                for mi_, m in enumerate(m_order):
                    g, slot = m // 6, m % 6
                    wq_sb = pa.tile([128, C], BF16, tag="wq", bufs=2, name="wq")
                    nc.sync.dma_start(wq_sb[:], wq_d[m])
                    big = pap.tile([128, 1024], F32, tag="big", bufs=2, name="big")
                    for ch in range(2):
                        for kk in range(NKC):
                            nc.tensor.matmul(
                                big[:, ch * 512:ch * 512 + CH],
                                wq_sb[:, kk * 128:(kk + 1) * 128],
                                n1[kk][:, ch, :],
                                start=(kk == 0), stop=(kk == NKC - 1))
                    pre = pa.tile([128, 1024], BF16, tag="pre", bufs=2, name="pre")
                    nc.scalar.copy(pre[:], big[:])
                    src = pre[:].rearrange("p (c n) -> p c n", c=2)
                    acc = pa.tile([128, 2, 256], F32, tag="acc", bufs=2, name="acc")
                    nc.scalar.activation(acc[:], src[:, :, 0:256], AF.Copy,
                                         scale=cw_sb[:, m * 4:m * 4 + 1])
                    for j in range(1, DCONV):
                        nc.vector.scalar_tensor_tensor(
                            acc[:], src[:, :, j:j + 256],
                            cw_sb[:, m * 4 + j:m * 4 + j + 1], acc[:],
                            op0=ALU.mult, op1=ALU.add)
                    sl = pa.tile([128, 512], BF16, tag="sl", bufs=3, name="sl")
                    nc.scalar.activation(
                        sl[:].rearrange("p (c n) -> p c n", c=2), acc[:], AF.Silu)
                    if DEBUG:
                        nc.sync.dma_start(dbg["sl"][m * 128:(m + 1) * 128, :], sl[:])

                    if slot <= 4:  # q heads and k: rope
                        # rot = [-x2; x1] via PE rotation matmul, then
                        # ro = sl*[c;c] + rot*[s;s]
                        rot_ps = pap.tile([128, 512], F32, tag="ps1", bufs=3, name="ps1")
                        nc.tensor.matmul(rot_ps[:], rotm[:], sl[:],
                                         start=True, stop=True)
                        tt1 = pa.tile([128, 512], BF16, tag="tt1", bufs=2, name="tt1")
                        nc.vector.tensor_mul(tt1[:], sl[:], trig_sb[:, 0:512])
                        tt2 = pa.tile([128, 512], BF16, tag="tt2", bufs=2, name="tt2")
                        nc.vector.tensor_mul(tt2[:], rot_ps[:], trig_sb[:, 512:1024])
                        ro = pa.tile([128, 512], BF16, tag="ro", bufs=3, name="ro")
                        nc.vector.tensor_add(ro[:], tt1[:], tt2[:])
                        if slot < 4:
                            h = g * QPK + slot
                            nc.sync.dma_start(
                                t1i_q[(h // 2) * 256 + (h % 2) * 128:
                                      (h // 2) * 256 + (h % 2) * 128 + 128, :],
                                ro[:])
                        else:  # k -> both consumer cores
                            for d in (2 * g, 2 * g + 1):
                                nc.sync.dma_start(
                                    t1i_kv[d * 256:d * 256 + 128, :], ro[:])
                    else:  # v: transpose to token-major (DMA xbar transpose)
                        for i in range(4):
                            vts = pa.tile([128, 128], BF16, tag="vts", bufs=3, name="vts")
                            nc.sync.dma_start_transpose(vts[:], sl[:, i * 128:(i + 1) * 128])
                            for d in (2 * g, 2 * g + 1):
                                vreg = t1i_kv[d * 256 + 128:d * 256 + 256, :] \
                                    .rearrange("p (a b) -> (p a) b", b=128)
                                nc.sync.dma_start(
                                    vreg[i * 128:(i + 1) * 128, :], vts[:])
                    if mi_ == 7:  # all kv tiles written -> fire kv exchange
                        nc.gpsimd.collective_compute(
                            "AllToAll", ALU.bypass,
                            replica_groups=[list(range(NCORES))],
                            ins=[t1i_kv[:].opt()], outs=[t1o_kv[:].opt()])

            nc.gpsimd.collective_compute(
                "AllToAll", ALU.bypass,
                replica_groups=[list(range(NCORES))],
                ins=[t1i_q[:].opt()], outs=[t1o_q[:].opt()])
            if DEBUG:
                nc.sync.dma_start(dbg["t1o"][0:2048, :], t1o_kv[:])
                nc.sync.dma_start(dbg["t1o"][2048:4096, :], t1o_q[:])

            # ============================================================
            # Phase B: head-parallel causal attention (2 heads per core)
            # ============================================================
            with tc.tile_pool(name="pb_sb", bufs=1) as pb, \
                 tc.tile_pool(name="pb_ps", bufs=1, space="PSUM") as pbp:
                y_t = [pb.tile([128, B * T], BF16, tag=f"y{i}", name=f"y{i}")
                       for i in range(2)]
                for hl in range(2):
                    rho_raw = pb.tile([8, 512], F32, tag="rho_raw", bufs=2, name="rho_raw")
                    osb_all = {}
                    for beta in range(B):
                        kall = pb.tile([128, 2048], BF16, tag="kall", bufs=2, name="kall")
                        vall = pb.tile([128, 16, 128], BF16, tag="vall", bufs=2, name="vall")
                        for kb in range(8):
                            jj = beta * 4 + kb // 2
                            pos = kb % 2
                            nc.sync.dma_start(
                                kall[:, kb * 256:(kb + 1) * 256],
                                t1o_kv[jj * 256:jj * 256 + 128,
                                       pos * 256:(pos + 1) * 256])
                            vreg = t1o_kv[jj * 256 + 128:jj * 256 + 256, :] \
                                .rearrange("p (a b) -> (p a) b", b=128)
                            for i in range(2):
                                nc.sync.dma_start(
                                    vall[:, kb * 2 + i, :],
                                    vreg[pos * 256 + i * 128:pos * 256 + (i + 1) * 128, :])
                        qall = pb.tile([128, 2048], BF16, tag="qall", bufs=2, name="qall")
                        for bq in range(8):
                            jj = beta * 4 + bq // 2
                            pos = bq % 2
                            nc.sync.dma_start(
                                qall[:, bq * 256:(bq + 1) * 256],
                                t1o_q[jj * 256 + hl * 128:jj * 256 + (hl + 1) * 128,
                                      pos * 256:(pos + 1) * 256])
                        for bp in range(4):
                            o_ps = pbp.tile([128, 512], F32, tag="o", bufs=2, name="o")
                            rs_ps = pbp.tile([128, 512], F32, tag="rs", bufs=2, name="rs")
                            nkb = 2 * bp + 2
                            for kb in range(nkb):
                                s_ps = pbp.tile([128, 2, 512], F32, tag="s", bufs=2, name="s")
                                p_sb = pb.tile([128, 2, 512], BF16, tag="p", bufs=4, name="p")
                                # column offsets: skip fully-masked tq ranges in
                                # the two diagonal key blocks of each 512-pair
                                if kb == nkb - 2:
                                    c0s, mof = (0, 128), 0
                                elif kb == nkb - 1:
                                    c0s, mof = (256, 384), 1024
                                else:
                                    c0s, mof = (0, 0), None
                                for i in range(2):
                                    c0 = c0s[i]
                                    nc.tensor.matmul(
                                        s_ps[:, i, c0:],
                                        kall[:, kb * 256 + i * 128:kb * 256 + (i + 1) * 128],
                                        qall[:, bp * 512 + c0:(bp + 1) * 512],
                                        start=True, stop=True)
                                if mof is None:
                                    nc.scalar.activation(p_sb[:], s_ps[:], AF.Exp,
                                                         scale=SCALE)
                                else:
                                    for i in range(2):
                                        c0 = c0s[i]
                                        nc.scalar.activation(
                                            p_sb[:, i, c0:], s_ps[:, i, c0:],
                                            AF.Exp, scale=SCALE)
                                        nc.vector.tensor_mul(
                                            p_sb[:, i, c0:], p_sb[:, i, c0:],
                                            msk_sb[:, mof + i * 512 + c0:
                                                   mof + (i + 1) * 512])
                                for i in range(2):
                                    c0 = c0s[i]
                                    nc.tensor.matmul(
                                        o_ps[:, c0:], vall[:, kb * 2 + i, :],
                                        p_sb[:, i, c0:],
                                        start=(kb == 0 and i == 0),
                                        stop=(kb == nkb - 1 and i == 1))
                                    nc.tensor.matmul(
                                        rs_ps[:, c0:], ones128[:],
                                        p_sb[:, i, c0:],
                                        start=(kb == 0 and i == 0),
                                        stop=(kb == nkb - 1 and i == 1))
                            ot = pb.tile([128, 512], BF16, tag=f"osb{beta}_{bp}",
                                         bufs=1, name=f"osb{beta}_{bp}")
                            nc.scalar.copy(ot[:], o_ps[:])
                            osb_all[(beta, bp)] = ot
                            rsrow = pb.tile([1, 512], F32, tag="rsrow", bufs=3, name="rsrow")
                            nc.scalar.copy(rsrow[:], rs_ps[0:1, :])
                            nc.sync.dma_start(
                                rho_raw[beta * 4 + bp:beta * 4 + bp + 1, :], rsrow[:])
                    rho = pb.tile([8, 512], BF16, tag="rho", bufs=2, name="rho")
                    with nc.allow_low_precision(reason="softmax denom in bf16"):
                        nc.vector.reciprocal(rho[:], rho_raw[:])
                    for beta in range(B):
                        for bp in range(4):
                            r_ = beta * 4 + bp
                            rhob_ps = pbp.tile([128, 512], F32, tag="s", bufs=2, name="rhob")
                            nc.tensor.matmul(rhob_ps[:],
                                             sel_sb[:, r_ * 128:(r_ + 1) * 128],
                                             rho[:], start=True, stop=True)
                            nc.vector.tensor_mul(
                                y_t[hl][:, beta * 2048 + bp * 512:
                                        beta * 2048 + (bp + 1) * 512],
                                osb_all[(beta, bp)][:], rhob_ps[:])
                    # this head-half is complete: exchange it while the other
                    # half computes
                    t2ih = t2i_a if hl == 0 else t2i_b
                    t2oh = t2o_a if hl == 0 else t2o_b
                    for j in range(8):
                        nc.sync.dma_start(
                            t2ih[j * 128:(j + 1) * 128, :],
                            y_t[hl][:, j * 512:(j + 1) * 512])
                    nc.gpsimd.collective_compute(
                        "AllToAll", ALU.bypass,
                        replica_groups=[list(range(NCORES))],
                        ins=[t2ih[:].opt()], outs=[t2oh[:].opt()])
                if DEBUG:
                    for hl in range(2):
                        nc.sync.dma_start(dbg["y"][hl * 128:(hl + 1) * 128, :],
                                          y_t[hl][:])

            # ============================================================
            # Phase C: proj + residual, norm2, MLP, output
            # ============================================================
            with tc.tile_pool(name="pc_sb", bufs=1) as pc_, \
                 tc.tile_pool(name="pc_ps", bufs=1, space="PSUM") as pcp:
                x2 = [pc_.tile([128, TOK], F32, tag=f"x2_{i}", name=f"x2_{i}")
                      for i in range(NKC)]
                n2 = [pc_.tile([128, TOK], BF16, tag=f"n2_{i}", name=f"n2_{i}")
                      for i in range(NKC)]
                h_t = [pc_.tile([128, TOK], BF16, tag=f"h{i}", name=f"h{i}")
                       for i in range(NMI)]
                with tc.tile_pool(name="pcy", bufs=1) as pcy:
                    kk_order = list(range(0, NKC, 2)) + list(range(1, NKC, 2))
                    for kk in kk_order:
                        src = t2o_a if kk % 2 == 0 else t2o_b
                        nc.sync.dma_start(yk[kk][:],
                                          src[(kk // 2) * 128:(kk // 2 + 1) * 128, :])
                    evens = kk_order[:8]
                    odds = kk_order[8:]
                    for base in range(0, 16, 5):
                        blk = range(base, min(base + 5, 16))
                        mm_tiles = {}
                        wp_tiles = {}
                        for mo in blk:
                            wp_sb = pcy.tile([128, C], BF16, tag="wpst", bufs=6, name="wpst")
                            nc.sync.dma_start(wp_sb[:], wp_d[mo])
                            wp_tiles[mo] = wp_sb
                            mm_ps = pcp.tile([128, TOK], F32, tag="mm", bufs=6, name="mm")
                            mm_tiles[mo] = mm_ps
                            for ik, kk in enumerate(evens):
                                nc.tensor.matmul(mm_ps[:],
                                                 wp_sb[:, kk * 128:(kk + 1) * 128],
                                                 yk[kk][:],
                                                 start=(ik == 0), stop=False)
                        for mo in blk:
                            for ik, kk in enumerate(odds):
                                nc.tensor.matmul(mm_tiles[mo][:],
                                                 wp_tiles[mo][:, kk * 128:(kk + 1) * 128],
                                                 yk[kk][:],
                                                 start=False, stop=(ik == len(odds) - 1))
                            nc.vector.tensor_add(x2[mo][:], xh[mo][:, HALO:], mm_tiles[mo][:])
                            if DEBUG:
                                nc.sync.dma_start(dbg["x2"][mo * 128:(mo + 1) * 128, :],
                                                  x2[mo][:])

                ss2 = pcp.tile([128, TOK], F32, tag="nrm", bufs=2, name="nrm")
                for kk in range(NKC):
                    x2sq = pc_.tile([128, TOK], BF16, tag="x2sq", bufs=3, name="x2sq")
                    nc.scalar.activation(x2sq[:], x2[kk][:], AF.Square)
                    nc.tensor.matmul(ss2[:], ones128[:], x2sq[:],
                                     start=(kk == 0), stop=(kk == NKC - 1))
                rt2 = pc_.tile([1, TOK], F32, tag="rt2", bufs=1, name="rt2")
                nc.scalar.activation(rt2[:], ss2[0:1, :], AF.Sqrt, bias=eps1[:], scale=1.0 / C)
                rinv2 = pc_.tile([1, TOK], F32, tag="rinv2", bufs=1, name="rinv2")
                nc.vector.reciprocal(rinv2[:], rt2[:])
                rb2 = pc_.tile([128, TOK], F32, tag="rb2", bufs=1, name="rb2")
                nc.gpsimd.partition_broadcast(rb2[:], rinv2[:])
                for kk in range(NKC):
                    nc.vector.tensor_mul(n2[kk][:], x2[kk][:], rb2[:])

                for mi in range(NMI):
                    w1_sb = pc_.tile([128, C], BF16, tag="wst", bufs=3, name="wst")
                    nc.sync.dma_start(w1_sb[:], w1_d[mi])
                    h1_ps = pcp.tile([128, TOK], F32, tag="mm", bufs=6, name="mm")
                    for kk in range(NKC):
                        nc.tensor.matmul(h1_ps[:],
                                         w1_sb[:, kk * 128:(kk + 1) * 128],
                                         n2[kk][:],
                                         start=(kk == 0), stop=(kk == NKC - 1))
                    s1 = pc_.tile([128, TOK], BF16, tag="s1", bufs=2, name="s1")
                    nc.scalar.activation(s1[:], h1_ps[:], AF.Silu)
                    w2_sb = pc_.tile([128, C], BF16, tag="wst", bufs=3, name="wst")
                    nc.sync.dma_start(w2_sb[:], w2_d[mi])
                    h2_ps = pcp.tile([128, TOK], F32, tag="mm", bufs=6, name="mm")
                    for kk in range(NKC):
                        nc.tensor.matmul(h2_ps[:],
                                         w2_sb[:, kk * 128:(kk + 1) * 128],
                                         n2[kk][:],
                                         start=(kk == 0), stop=(kk == NKC - 1))
                    nc.vector.tensor_mul(h_t[mi][:], s1[:], h2_ps[:])

                with tc.tile_pool(name="pcm", bufs=1) as pcm:
                    for mo in range(16):
                        wm_sb = pcm.tile([128, IM], BF16, tag="wm", bufs=2, name="wm")
                        nc.sync.dma_start(wm_sb[:], wm_d[mo])
                        mp_ps = pcp.tile([128, TOK], F32, tag="mm", bufs=6, name="mm")
                        for ki in range(NMI):
                            nc.tensor.matmul(mp_ps[:],
                                             wm_sb[:, ki * 128:(ki + 1) * 128],
                                             h_t[ki][:],
                                             start=(ki == 0), stop=(ki == NMI - 1))
                        outsb = pc_.tile([128, TOK], F32, tag="outsb", bufs=2, name="outsb")
                        nc.vector.tensor_add(outsb[:], x2[mo][:], mp_ps[:])
                        nc.sync.dma_start(out_d[mo * 128:(mo + 1) * 128, :], outsb[:])

    nc.compile()
    return nc


# --------------------------------------------------------------------------
# host-side prep / gather
# --------------------------------------------------------------------------

def _prep_lhsT(w, nm, nk):
    """w: (out, in) f32 -> (nm, 128, nk*128) bf16 where
    prep[m][p][k*128+c] = w[m*128+c, k*128+p]."""
    o, i = w.shape
    assert o == nm * 128 and i == nk * 128
    r = w.reshape(nm, 128, nk, 128).transpose(0, 3, 2, 1)  # (m, p, k, c)
    return np.ascontiguousarray(r.reshape(nm, 128, nk * 128)).astype(ml_dtypes.bfloat16)


def _host_inputs(inputs):
    x = np.asarray(inputs["x"], np.float32)          # (B, T, C)
    cos = np.asarray(inputs["cos"], np.float32)      # (T, 64)
    sin = np.asarray(inputs["sin"], np.float32)
    n1w = np.asarray(inputs["norm1_w"], np.float32)
    n2w = np.asarray(inputs["norm2_w"], np.float32)

    # fold rmsnorm weights into the (pre-transposed) weight matrices
    attn_w = np.asarray(inputs["attn_w"], np.float32) * n1w[None, :]
    fc1_w = np.asarray(inputs["fc1_w"], np.float32) * n2w[None, :]
    fc2_w = np.asarray(inputs["fc2_w"], np.float32) * n2w[None, :]
    proj_w = np.asarray(inputs["proj_w"], np.float32)
    mlp_w = np.asarray(inputs["mlp_proj_w"], np.float32)

    wq = _prep_lhsT(attn_w, NMQ, NKC)
    wp = _prep_lhsT(proj_w, 16, NKC)
    w1 = _prep_lhsT(fc1_w, NMI, NKC)
    w2 = _prep_lhsT(fc2_w, NMI, NKC)
    wm = _prep_lhsT(mlp_w, 16, NMI)

    # conv weights in qkv m-tile order: per g: q0..q3 (qconv), k, v
    cw = np.zeros((NMQ, 128, DCONV), np.float32)
    qc = np.asarray(inputs["qconv_w"], np.float32)
    kc = np.asarray(inputs["kconv_w"], np.float32)
    vc = np.asarray(inputs["vconv_w"], np.float32)
    for g in range(NG):
        for s in range(QPK):
            cw[g * 6 + s] = qc[(g * QPK + s) * 128:(g * QPK + s + 1) * 128]
        cw[g * 6 + 4] = kc[g * 128:(g + 1) * 128]
        cw[g * 6 + 5] = vc[g * 128:(g + 1) * 128]
    cw = np.ascontiguousarray(cw.transpose(1, 0, 2).reshape(128, NMQ * DCONV))

    # paired-block diag masks, each (128, 2, 512) flattened to (128, 1024):
    # mskA: kb == nkb-2 (tk rel = i*128+p); mskB: kb == nkb-1 (tk rel = 256+i*128+p)
    p = np.arange(128)[:, None]
    f = np.arange(512)[None, :]
    mskA = np.concatenate([(p <= f), (p + 128 <= f)], axis=1)
    mskB = np.concatenate([(p + 256 <= f), (p + 384 <= f)], axis=1)
    msk = np.concatenate([mskA, mskB], axis=1).astype(np.float32)
    msk = msk.astype(ml_dtypes.bfloat16)

    # rho-broadcast selectors: sel[:, bq*128:(bq+1)*128] one-hot row bq
    sel = np.zeros((8, 1024), np.float32)
    for bq in range(8):
        sel[bq, bq * 128:(bq + 1) * 128] = 1.0
    sel = sel.astype(ml_dtypes.bfloat16)

    # rope rotation: rot = rotm.T @ x = [-x2; x1]
    rotm = np.zeros((128, 128), np.float32)
    for m in range(64):
        rotm[m + 64, m] = -1.0
        rotm[m, m + 64] = 1.0
    rotm = rotm.astype(ml_dtypes.bfloat16)

    # per-core x (feature-major with halo) and trig
    xt = x.transpose(0, 2, 1)                        # (B, C, T)
    xpad = np.concatenate([np.zeros((B, C, HALO), np.float32), xt], axis=2)
    cosT = cos.T                                     # (64, T)
    sinT = sin.T
    in_maps = []
    for c in range(NCORES):
        beta, tb = c // 4, (512 * c) % 2048
        xc = np.ascontiguousarray(xpad[beta, :, tb:tb + XW])
        cs = np.concatenate([cosT[:, tb:tb + TOK], cosT[:, tb:tb + TOK]], axis=0)
        ss = np.concatenate([sinT[:, tb:tb + TOK], sinT[:, tb:tb + TOK]], axis=0)
        trig = np.concatenate([cs, ss], axis=1).astype(ml_dtypes.bfloat16)
        in_maps.append({
            "x": xc, "wq": wq, "wp": wp, "w1": w1, "w2": w2, "wm": wm,
            "cw": cw, "trig": np.ascontiguousarray(trig), "msk": msk, "sel": sel,
            "rotm": rotm,
        })
    return in_maps


_NC_CACHE = None


def kernel(**inputs) -> np.ndarray:
    global LAST_RESULTS, _NC_CACHE
    if _NC_CACHE is None:
        _NC_CACHE = build_nc()
    nc = _NC_CACHE
    in_maps = _host_inputs(inputs)
    res = run_bass_kernel_spmd(nc, in_maps, list(range(NCORES)), trace=TRACE)
    LAST_RESULTS = res
    out = np.zeros((B, T, C), np.float32)
    for c in range(NCORES):
        oc = res.results[c]["out"]                   # (C, TOK) feature-major
        beta, tb = c // 4, (512 * c) % 2048
        out[beta, tb:tb + TOK, :] = oc.T
    return out



# revision 8
# speedup vs baseline: 1.1742x; 1.1742x over previous
"""Distributed Trainium2 kernel for the dense transformer block.

Strategy (8 NeuronCores, SPMD), v2 — (kv-group x batch)-parallel attention:
  Core c handles query group g = c//2 of batch beta = c%2 for ALL 2048
  tokens. qkv projection (fp8 DoubleRow matmuls), depthwise causal conv,
  SiLU and RoPE are fully local (no halo, no collective). Causal
  flash-attention for the core's 4 heads is fully local too: scores in
  bf16, exp written straight to fp8, AV and rowsum as fp8 DoubleRow
  matmuls (2x PE rate).
  One small AllToAll per head (fp8, 256KB) reshards y to token-parallel
  (each core: 256 tokens of each batch), overlapped with the next head's
  attention.
  Phase C (token-parallel): proj in fp8 DoubleRow + residual -> rmsnorm2
  -> gated MLP in bf16 (fp8 fails the precision budget there) ->
  residual. Output is feature-major (2048, 512) per core.

All fp8 operands use a fixed scale of 32 (values here are < 6 in
magnitude; fp8e4 clips at 240). Weight tensors are quantized per output
channel on the host; dequant scales fold into the PSUM->SBUF copies.
"""
import os
import sys

sys.path.insert(0, "/opt/trn_rl_repo")

import numpy as np
import ml_dtypes

import concourse.bass as bass
import concourse.mybir as mybir
from concourse import bacc, tile
from concourse.bass_utils import run_bass_kernel_spmd

B, T, C = 2, 2048, 2048
NH, NG, HS = 16, 4, 128
QPK = NH // NG
DCONV = 4
IM = 5632
EPS = 1e-5
NCORES = 8
NKC = C // 128       # 16
NMI = IM // 128      # 44
TOK = 512            # phase-C tokens per core (256 of each batch)
SCALE = 1.0 / float(np.sqrt(HS))
QS = 32.0            # fp8 activation scale
LN_QS = float(np.log(QS))

F32 = mybir.dt.float32
BF16 = mybir.dt.bfloat16
FP8 = mybir.dt.float8e4
AF = mybir.ActivationFunctionType
ALU = mybir.AluOpType
PM = mybir.MatmulPerfMode

DEBUG = bool(int(os.environ.get("KERNEL_DEBUG", "0")))
TRACE = bool(int(os.environ.get("KERNEL_TRACE", "0")))

LAST_RESULTS = None  # test.py reads exec_time from here


# --------------------------------------------------------------------------
# builder
# --------------------------------------------------------------------------

def build_nc():
    nc = bacc.Bacc("TRN2", target_bir_lowering=False, debug=False,
                   enable_asserts=True, num_devices=NCORES)

    # per-core inputs
    x8_d = nc.dram_tensor("x8", [128, NKC, T], FP8, kind="ExternalInput")
    xc_d = nc.dram_tensor("xc", [C, TOK], F32, kind="ExternalInput")
    wq_d = nc.dram_tensor("wq", [6, 128, NKC, 128], FP8, kind="ExternalInput")
    tmq_d = nc.dram_tensor("tmq", [128, 6], F32, kind="ExternalInput")
    wp_d = nc.dram_tensor("wp", [16, 128, NKC, 128], FP8, kind="ExternalInput")
    tmp_d = nc.dram_tensor("tmp", [128, 16], F32, kind="ExternalInput")
    w1_d = nc.dram_tensor("w1", [NMI, 128, C], BF16, kind="ExternalInput")
    w2_d = nc.dram_tensor("w2", [NMI, 128, C], BF16, kind="ExternalInput")
    wm_d = nc.dram_tensor("wm", [16, 128, IM], BF16, kind="ExternalInput")
    cw_d = nc.dram_tensor("cw", [128, 6 * DCONV], F32, kind="ExternalInput")
    trig_d = nc.dram_tensor("trig", [128, 2 * T], BF16, kind="ExternalInput")
    msk_d = nc.dram_tensor("msk", [128, 2048], BF16, kind="ExternalInput")
    rotm_d = nc.dram_tensor("rotm", [128, 128], BF16, kind="ExternalInput")
    out_d = nc.dram_tensor("out", [C, TOK], F32, kind="ExternalOutput")

    # collective buffers: per head, chunk j goes to / comes from core j
    t2i = [nc.dram_tensor(f"t2i{h}", [NCORES, 128, 256], FP8, kind="Internal")
           for h in range(QPK)]
    t2o = [nc.dram_tensor(f"t2o{h}", [NCORES, 128, 256], FP8, kind="Internal")
           for h in range(QPK)]

    dbg = {}
    if DEBUG:
        dbg["sl"] = nc.dram_tensor("d_sl", [6 * 128, T], BF16, kind="ExternalOutput")
        dbg["y8"] = nc.dram_tensor("d_y8", [QPK * 128, T], BF16, kind="ExternalOutput")
        dbg["x2"] = nc.dram_tensor("d_x2", [C, TOK], F32, kind="ExternalOutput")
        dbg["rinv"] = nc.dram_tensor("d_rinv", [1, T], F32, kind="ExternalOutput")

    with tile.TileContext(nc) as tc:
        with tc.tile_pool(name="pers", bufs=1) as pers:
            # ---- constants ----
            cw_sb = pers.tile([128, 6 * DCONV], F32, tag="cw", name="cw")
            tmq_sb = pers.tile([128, 6], F32, tag="tmq", name="tmq")
            tmp_sb = pers.tile([128, 16], F32, tag="tmp", name="tmp")
            nc.sync.dma_start(cw_sb[:], cw_d[:])
            nc.sync.dma_start(tmq_sb[:], tmq_d[:])
            nc.sync.dma_start(tmp_sb[:], tmp_d[:])

            ones128 = pers.tile([128, 128], BF16, tag="ones128", name="ones128")
            ones8 = pers.tile([128, 2, 128], FP8, tag="ones8", name="ones8")
            eps1 = pers.tile([1, 1], F32, tag="eps1", name="eps1")
            lnq = pers.tile([128, 1], F32, tag="lnq", name="lnq")
            nc.gpsimd.memset(ones128[:], 1.0)
            nc.gpsimd.memset(ones8[:], 1.0)
            nc.gpsimd.memset(eps1[:], EPS)
            nc.gpsimd.memset(lnq[:], LN_QS)

            # pool spanning phases A+B (closed before C to free SBUF)
            pab_cm = tc.tile_pool(name="pab_sb", bufs=1)
            pab = pab_cm.__enter__()
            msk_sb = pab.tile([128, 2048], BF16, tag="msk", name="msk")
            nc.sync.dma_start(msk_sb[:], msk_d[:])
            # attention activations, persist from A into B
            qall = [pab.tile([128, T], BF16, tag=f"q{h}", name=f"q{h}")
                    for h in range(QPK)]
            kall = pab.tile([128, T], BF16, tag="kall", name="kall")
            v8t = pab.tile([128, NKC, 128], FP8, tag="v8t", name="v8t")

            # ============================================================
            # Phase A: rmsnorm1 (scale only) -> qkv (fp8 DR) -> conv ->
            #          silu -> rope
            # ============================================================
            with tc.tile_pool(name="pa_sb", bufs=1) as pa, \
                 tc.tile_pool(name="pa_ps", bufs=1, space="PSUM") as pap:
                trig_sb = pa.tile([128, 2 * T], BF16, tag="trig", name="trig")
                rotm = pa.tile([128, 128], BF16, tag="rotm", name="rotm")
                nc.sync.dma_start(trig_sb[:], trig_d[:])
                nc.sync.dma_start(rotm[:], rotm_d[:])
                # x8: core's batch, feature-major [p, kk, t], value 32*x
                x8 = pa.tile([128, NKC, T], FP8, tag="x8", name="x8")
                for qa in range(4):
                    nc.sync.dma_start(x8[:, qa * 4:(qa + 1) * 4, :],
                                      x8_d[:, qa * 4:(qa + 1) * 4, :])
                rinvb = pa.tile([128, T], F32, tag="rinvb", name="rinvb")
                # per tq block: sum of squares over all features
                for tq in range(4):
                    ss_ps = pap.tile([128, 512], F32, tag="ss", bufs=2, name="ss")
                    for kk in range(NKC):
                        xsq = pa.tile([128, 512], BF16, tag="xsq", bufs=3, name="xsq")
                        nc.scalar.activation(
                            xsq[:], x8[:, kk, tq * 512:(tq + 1) * 512], AF.Square)
                        nc.tensor.matmul(ss_ps[:], ones128[:], xsq[:],
                                         start=(kk == 0), stop=(kk == NKC - 1))
                    # ss = 1024 * sum(x^2); rt = sqrt(mean + eps)
                    rt = pa.tile([1, 512], F32, tag="rt", bufs=2, name="rt")
                    nc.scalar.activation(rt[:], ss_ps[0:1, :], AF.Sqrt,
                                         bias=eps1[:], scale=1.0 / (C * QS * QS))
                    rinv = pa.tile([1, 512], F32, tag="rinv", bufs=2, name="rinv")
                    nc.vector.reciprocal(rinv[:], rt[:])
                    nc.gpsimd.partition_broadcast(
                        rinvb[:, tq * 512:(tq + 1) * 512], rinv[:])
                if DEBUG:
                    nc.sync.dma_start(dbg["rinv"][:], rinvb[0:1, :])

                # qkv: local m-tiles 0..5 = q0..q3, k, v of this group
                pre = {}
                for m in range(6):
                    pre[m] = pa.tile([128, DCONV - 1 + T], BF16,
                                     tag=f"pre{m}", name=f"pre{m}")
                    nc.gpsimd.memset(pre[m][:, 0:DCONV - 1], 0.0)
                wq_sb = [pa.tile([128, NKC, 128], FP8, tag=f"wq{m}", name=f"wq{m}")
                         for m in range(6)]
                for m in range(6):
                    nc.sync.dma_start(wq_sb[m][:], wq_d[m])
                for m in range(6):
                    for tq in range(4):
                        qk_ps = pap.tile([128, 512], F32, tag="qk", bufs=3, name="qk")
                        for jp in range(NKC // 2):
                            nc.tensor.matmul(
                                qk_ps[:],
                                wq_sb[m][:, 2 * jp:2 * jp + 2, :],
                                x8[:, 2 * jp:2 * jp + 2, tq * 512:(tq + 1) * 512],
                                start=(jp == 0), stop=(jp == NKC // 2 - 1),
                                perf_mode=PM.DoubleRow)
                        # dequant: per-channel weight scale * per-token rinv
                        nc.vector.scalar_tensor_tensor(
                            pre[m][:, DCONV - 1 + tq * 512:DCONV - 1 + (tq + 1) * 512],
                            qk_ps[:], tmq_sb[:, m:m + 1],
                            rinvb[:, tq * 512:(tq + 1) * 512],
                            op0=ALU.mult, op1=ALU.mult)

                # conv + silu (+ rope for q/k, fp8 cast for v)
                for m in range(6):
                    acc = pa.tile([128, T], F32, tag="acc", bufs=2, name="acc")
                    nc.scalar.activation(acc[:], pre[m][:, 0:T], AF.Copy,
                                         scale=cw_sb[:, m * 4:m * 4 + 1])
                    for j in range(1, DCONV):
                        nc.vector.scalar_tensor_tensor(
                            acc[:], pre[m][:, j:j + T],
                            cw_sb[:, m * 4 + j:m * 4 + j + 1], acc[:],
                            op0=ALU.mult, op1=ALU.add)
                    if m < 5:  # q heads and k: silu then rope
                        sl = pa.tile([128, T], BF16, tag="sl", bufs=2, name="sl")
                        nc.scalar.activation(sl[:], acc[:], AF.Silu)
                        if DEBUG:
                            nc.sync.dma_start(dbg["sl"][m * 128:(m + 1) * 128, :], sl[:])
                        dst = qall[m][:] if m < 4 else kall[:]
                        tt1 = pa.tile([128, T], BF16, tag="tt1", bufs=2, name="tt1")
                        nc.vector.tensor_mul(tt1[:], sl[:], trig_sb[:, 0:T])
                        for tq in range(4):
                            rot_ps = pap.tile([128, 512], F32, tag="rot", bufs=2,
                                              name="rot")
                            nc.tensor.matmul(rot_ps[:], rotm[:],
                                             sl[:, tq * 512:(tq + 1) * 512],
                                             start=True, stop=True)
                            tt2 = pa.tile([128, 512], BF16, tag="tt2", bufs=3,
                                          name="tt2")
                            nc.vector.tensor_mul(
                                tt2[:], rot_ps[:],
                                trig_sb[:, T + tq * 512:T + (tq + 1) * 512])
                            nc.vector.tensor_add(
                                dst[:, tq * 512:(tq + 1) * 512],
                                tt1[:, tq * 512:(tq + 1) * 512], tt2[:])
                    else:  # v: silu -> transpose -> fp8 (x32)
                        vsl = pa.tile([128, T], BF16, tag="vsl", bufs=1, name="vsl")
                        nc.scalar.activation(vsl[:], acc[:], AF.Silu)
                        if DEBUG:
                            nc.sync.dma_start(dbg["sl"][5 * 128:6 * 128, :], vsl[:])
                        vt = pa.tile([128, NKC, 128], BF16, tag="vt", name="vt")
                        for i in range(NKC):
                            nc.sync.dma_start_transpose(
                                vt[:, i, :], vsl[:, i * 128:(i + 1) * 128])
                        with nc.allow_low_precision(reason="fp8 quantize v"):
                            nc.scalar.activation(v8t[:], vt[:], AF.Copy, scale=QS)

            # ============================================================
            # Phase B: causal attention, 4 local heads; per-head A2A of y8
            # ============================================================
            with tc.tile_pool(name="pb_sb", bufs=1) as pb, \
                 tc.tile_pool(name="pb_ps", bufs=1, space="PSUM") as pbp:
                for h in range(QPK):
                    y8 = pb.tile([128, T], FP8, tag="y8", bufs=2, name="y8")
                    for bp in range(4):
                        npair = 2 * (bp + 1)
                        o_ps = pbp.tile([128, 512], F32, tag="o", bufs=2, name="o")
                        rs_ps = pbp.tile([128, 512], F32, tag="rs", bufs=1, name="rs")
                        p8s = []
                        for jp in range(npair):
                            # scores for pair jp (tk = jp*256 .. +256)
                            s_ps = pbp.tile([128, 2, 512], F32, tag="s", bufs=2,
                                            name="s")
                            for i in range(2):
                                nc.tensor.matmul(
                                    s_ps[:, i, :],
                                    kall[:, (jp * 2 + i) * 128:(jp * 2 + i + 1) * 128],
                                    qall[h][:, bp * 512:(bp + 1) * 512],
                                    start=True, stop=True)
                            p8 = pb.tile([128, 2, 512], FP8, tag="p8", bufs=4,
                                         name="p8")
                            diag = jp >= npair - 2
                            with nc.allow_low_precision(reason="fp8 softmax probs"):
                                if not diag:
                                    # p8 = 32 * exp(s * SCALE)
                                    nc.scalar.activation(p8[:], s_ps[:], AF.Exp,
                                                         bias=lnq[:], scale=SCALE)
                                else:
                                    pd = pb.tile([128, 2, 512], BF16, tag="pd",
                                                 bufs=2, name="pd")
                                    nc.scalar.activation(pd[:], s_ps[:], AF.Exp,
                                                         bias=lnq[:], scale=SCALE)
                                    mof = 0 if jp == npair - 2 else 1024
                                    nc.vector.tensor_mul(
                                        p8[:].rearrange("p a b -> p (a b)"),
                                        pd[:].rearrange("p a b -> p (a b)"),
                                        msk_sb[:, mof:mof + 1024])
                            p8s.append(p8)
                            # software pipeline: AV/rowsum one pair behind
                            if jp > 0:
                                pprev = p8s[jp - 1]
                                nc.tensor.matmul(
                                    o_ps[:], v8t[:, 2 * (jp - 1):2 * jp, :],
                                    pprev[:], start=(jp == 1), stop=False,
                                    perf_mode=PM.DoubleRow)
                                nc.tensor.matmul(
                                    rs_ps[:], ones8[:], pprev[:],
                                    start=(jp == 1), stop=False,
                                    perf_mode=PM.DoubleRow)
                        plast = p8s[npair - 1]
                        nc.tensor.matmul(
                            o_ps[:], v8t[:, 2 * (npair - 1):2 * npair, :],
                            plast[:], start=False, stop=True,
                            perf_mode=PM.DoubleRow)
                        nc.tensor.matmul(
                            rs_ps[:], ones8[:], plast[:],
                            start=False, stop=True, perf_mode=PM.DoubleRow)
                        # y8 = o / rowsum  (the 32s cancel; result is 32*y)
                        rho = pb.tile([1, 512], F32, tag="rho", bufs=2, name="rho")
                        nc.vector.reciprocal(rho[:], rs_ps[0:1, :])
                        rhob = pb.tile([128, 512], F32, tag="rhob", bufs=2,
                                       name="rhob")
                        nc.gpsimd.partition_broadcast(rhob[:], rho[:])
                        with nc.allow_low_precision(reason="fp8 y"):
                            nc.vector.tensor_mul(
                                y8[:, bp * 512:(bp + 1) * 512], o_ps[:], rhob[:])
                    if DEBUG:
                        y8b = pb.tile([128, T], BF16, tag="y8b", bufs=1, name="y8b")
                        nc.scalar.copy(y8b[:], y8[:])
                        nc.sync.dma_start(dbg["y8"][h * 128:(h + 1) * 128, :], y8b[:])
                    # resharding A2A for this head: chunk j -> core j
                    for j in range(NCORES):
                        nc.sync.dma_start(t2i[h][j], y8[:, 256 * j:256 * (j + 1)])
                    nc.gpsimd.collective_compute(
                        "AllToAll", ALU.bypass,
                        replica_groups=[list(range(NCORES))],
                        ins=[t2i[h][:].opt()], outs=[t2o[h][:].opt()])

            pab_cm.__exit__(None, None, None)

            # ============================================================
            # Phase C: proj (fp8 DR) + residual, norm2, MLP (bf16), output
            # ============================================================
            with tc.tile_pool(name="pc_sb", bufs=1) as pc_, \
                 tc.tile_pool(name="pc_ps", bufs=1, space="PSUM") as pcp:
                x2 = [pc_.tile([128, TOK], F32, tag=f"x2_{i}", name=f"x2_{i}")
                      for i in range(NKC)]
                n2 = [pc_.tile([128, TOK], BF16, tag=f"n2_{i}", name=f"n2_{i}")
                      for i in range(NKC)]
                h_t = [pc_.tile([128, TOK], BF16, tag=f"h{i}", name=f"h{i}")
                       for i in range(NMI)]

                with tc.tile_pool(name="pc0", bufs=1) as pc0:
                    xc = [pc0.tile([128, TOK], F32, tag=f"xc{i}", name=f"xc{i}")
                          for i in range(NKC)]
                    for kk in range(NKC):
                        nc.sync.dma_start(xc[kk][:], xc_d[kk * 128:(kk + 1) * 128, :])
                    # gather y8 into kk-major [p, kk, tok] (kk = g'*4 + h)
                    ysb = pc0.tile([128, NKC, TOK], FP8, tag="ysb", name="ysb")
                    for h in range(QPK):
                        for gp in range(4):
                            for b in range(2):
                                nc.sync.dma_start(
                                    ysb[:, gp * 4 + h, b * 256:(b + 1) * 256],
                                    t2o[h][2 * gp + b])

                    wp_sb = [pc0.tile([128, NKC, 128], FP8, tag=f"wp{mo}",
                                      name=f"wp{mo}") for mo in range(16)]
                    for mo in range(16):
                        nc.sync.dma_start(wp_sb[mo][:], wp_d[mo])
                    for mo in range(16):
                        mm_ps = pcp.tile([128, TOK], F32, tag="mm", bufs=6, name="mm")
                        for jp in range(NKC // 2):
                            nc.tensor.matmul(
                                mm_ps[:], wp_sb[mo][:, 2 * jp:2 * jp + 2, :],
                                ysb[:, 2 * jp:2 * jp + 2, :],
                                start=(jp == 0), stop=(jp == NKC // 2 - 1),
                                perf_mode=PM.DoubleRow)
                        # x2 = x + proj (per-channel dequant)
                        nc.vector.scalar_tensor_tensor(
                            x2[mo][:], mm_ps[:], tmp_sb[:, mo:mo + 1], xc[mo][:],
                            op0=ALU.mult, op1=ALU.add)
                        if DEBUG:
                            nc.sync.dma_start(dbg["x2"][mo * 128:(mo + 1) * 128, :],
                                              x2[mo][:])

                ss2 = pcp.tile([128, TOK], F32, tag="nrm", bufs=2, name="nrm")
                for kk in range(NKC):
                    x2sq = pc_.tile([128, TOK], BF16, tag="x2sq", bufs=3, name="x2sq")
                    nc.scalar.activation(x2sq[:], x2[kk][:], AF.Square)
                    nc.tensor.matmul(ss2[:], ones128[:], x2sq[:],
                                     start=(kk == 0), stop=(kk == NKC - 1))
                rt2 = pc_.tile([1, TOK], F32, tag="rt2", bufs=1, name="rt2")
                nc.scalar.activation(rt2[:], ss2[0:1, :], AF.Sqrt, bias=eps1[:],
                                     scale=1.0 / C)
                rinv2 = pc_.tile([1, TOK], F32, tag="rinv2", bufs=1, name="rinv2")
                nc.vector.reciprocal(rinv2[:], rt2[:])
                rb2 = pc_.tile([128, TOK], F32, tag="rb2", bufs=1, name="rb2")
                nc.gpsimd.partition_broadcast(rb2[:], rinv2[:])
                for kk in range(NKC):
                    nc.vector.tensor_mul(n2[kk][:], x2[kk][:], rb2[:])

                for mi in range(NMI):
                    w1_sb = pc_.tile([128, C], BF16, tag="wst", bufs=3, name="wst")
                    nc.sync.dma_start(w1_sb[:], w1_d[mi])
                    h1_ps = pcp.tile([128, TOK], F32, tag="mm", bufs=6, name="mm")
                    for kk in range(NKC):
                        nc.tensor.matmul(h1_ps[:],
                                         w1_sb[:, kk * 128:(kk + 1) * 128],
                                         n2[kk][:],
                                         start=(kk == 0), stop=(kk == NKC - 1))
                    s1 = pc_.tile([128, TOK], BF16, tag="s1", bufs=2, name="s1")
                    nc.scalar.activation(s1[:], h1_ps[:], AF.Silu)
                    w2_sb = pc_.tile([128, C], BF16, tag="wst", bufs=3, name="wst")
                    nc.sync.dma_start(w2_sb[:], w2_d[mi])
                    h2_ps = pcp.tile([128, TOK], F32, tag="mm", bufs=6, name="mm")
                    for kk in range(NKC):
                        nc.tensor.matmul(h2_ps[:],
                                         w2_sb[:, kk * 128:(kk + 1) * 128],
                                         n2[kk][:],
                                         start=(kk == 0), stop=(kk == NKC - 1))
                    nc.vector.tensor_mul(h_t[mi][:], s1[:], h2_ps[:])

                with tc.tile_pool(name="pcm", bufs=1) as pcm:
                    for mo in range(16):
                        wm_sb = pcm.tile([128, IM], BF16, tag="wm", bufs=2, name="wm")
                        nc.sync.dma_start(wm_sb[:], wm_d[mo])
                        mp_ps = pcp.tile([128, TOK], F32, tag="mm", bufs=6, name="mm")
                        for ki in range(NMI):
                            nc.tensor.matmul(mp_ps[:],
                                             wm_sb[:, ki * 128:(ki + 1) * 128],
                                             h_t[ki][:],
                                             start=(ki == 0), stop=(ki == NMI - 1))
                        outsb = pc_.tile([128, TOK], F32, tag="outsb", bufs=2,
                                         name="outsb")
                        nc.vector.tensor_add(outsb[:], x2[mo][:], mp_ps[:])
                        nc.sync.dma_start(out_d[mo * 128:(mo + 1) * 128, :], outsb[:])

    nc.compile()
    return nc


# --------------------------------------------------------------------------
# host-side prep / gather
# --------------------------------------------------------------------------

def _q8(a):
    return np.clip(a, -240.0, 240.0).astype(ml_dtypes.float8_e4m3)


def _prep_fp8_lhsT(w, nm, nk):
    """w: (out, in) f32 -> (lhsT fp8 [nm,128,nk,128], scales f32 [128,nm])
    with per-output-channel absmax quantization. Dequant scale includes
    the 1/QS for the fp8 rhs activations."""
    o, i = w.shape
    assert o == nm * 128 and i == nk * 128
    r = w.reshape(nm, 128, nk, 128).transpose(0, 3, 2, 1)  # (m, p, k, c)
    amax = np.abs(r).max(axis=(1, 2))                      # (m, c)
    amax = np.maximum(amax, 1e-30)
    q = _q8(r * (240.0 / amax[:, None, None, :]))
    scales = np.ascontiguousarray((amax / (240.0 * QS)).T).astype(np.float32)
    return np.ascontiguousarray(q), scales


def _prep_lhsT(w, nm, nk):
    """w: (out, in) f32 -> (nm, 128, nk*128) bf16 where
    prep[m][p][k*128+c] = w[m*128+c, k*128+p]."""
    o, i = w.shape
    assert o == nm * 128 and i == nk * 128
    r = w.reshape(nm, 128, nk, 128).transpose(0, 3, 2, 1)
    return np.ascontiguousarray(r.reshape(nm, 128, nk * 128)).astype(ml_dtypes.bfloat16)


def _host_inputs(inputs):
    x = np.asarray(inputs["x"], np.float32)          # (B, T, C)
    cos = np.asarray(inputs["cos"], np.float32)      # (T, 64)
    sin = np.asarray(inputs["sin"], np.float32)
    n1w = np.asarray(inputs["norm1_w"], np.float32)
    n2w = np.asarray(inputs["norm2_w"], np.float32)

    attn_w = np.asarray(inputs["attn_w"], np.float32) * n1w[None, :]
    fc1_w = np.asarray(inputs["fc1_w"], np.float32) * n2w[None, :]
    fc2_w = np.asarray(inputs["fc2_w"], np.float32) * n2w[None, :]
    proj_w = np.asarray(inputs["proj_w"], np.float32)
    mlp_w = np.asarray(inputs["mlp_proj_w"], np.float32)

    wq_all, tmq_all = _prep_fp8_lhsT(attn_w, NH + 2 * NG, NKC)  # (24,128,16,128)
    wp, tmp_s = _prep_fp8_lhsT(proj_w, 16, NKC)
    w1 = _prep_lhsT(fc1_w, NMI, NKC)
    w2 = _prep_lhsT(fc2_w, NMI, NKC)
    wm = _prep_lhsT(mlp_w, 16, NMI)

    qc = np.asarray(inputs["qconv_w"], np.float32)
    kc = np.asarray(inputs["kconv_w"], np.float32)
    vc = np.asarray(inputs["vconv_w"], np.float32)

    # masks: mskA for pair npair-2 (tk rel = i*128+p), mskB for npair-1
    p = np.arange(128)[:, None]
    f = np.arange(512)[None, :]
    mskA = np.concatenate([(p <= f), (p + 128 <= f)], axis=1)
    mskB = np.concatenate([(p + 256 <= f), (p + 384 <= f)], axis=1)
    msk = np.concatenate([mskA, mskB], axis=1).astype(np.float32)
    msk = msk.astype(ml_dtypes.bfloat16)

    rotm = np.zeros((128, 128), np.float32)
    for m in range(64):
        rotm[m + 64, m] = -1.0
        rotm[m, m + 64] = 1.0
    rotm = rotm.astype(ml_dtypes.bfloat16)

    # trig [128, 2T]: cols 0:T cos (64-halves stacked), T:2T sin
    cosT = cos.T                                     # (64, T)
    sinT = sin.T
    cs = np.concatenate([cosT, cosT], axis=0)        # (128, T)
    ss = np.concatenate([sinT, sinT], axis=0)
    trig = np.ascontiguousarray(
        np.concatenate([cs, ss], axis=1)).astype(ml_dtypes.bfloat16)

    # x8 per batch: [128, NKC, T] with x8[p,kk,t] = q8(32*x[beta,t,kk*128+p])
    xt = x.transpose(0, 2, 1)                        # (B, C, T)
    x8b = []
    for beta in range(B):
        a = xt[beta].reshape(NKC, 128, T).transpose(1, 0, 2)  # (128, NKC, T)
        x8b.append(np.ascontiguousarray(_q8(a * QS)))

    in_maps = []
    for c in range(NCORES):
        g, beta = c // 2, c % 2
        # local qkv m-tiles: q0..q3, k, v of group g
        msel = [g * 6 + s for s in range(6)]
        wq = np.ascontiguousarray(wq_all[msel])
        tmq = np.ascontiguousarray(tmq_all[:, msel])
        cw = np.zeros((128, 6 * DCONV), np.float32)
        for s in range(QPK):
            cw[:, s * DCONV:(s + 1) * DCONV] = qc[(g * QPK + s) * 128:(g * QPK + s + 1) * 128]
        cw[:, 4 * DCONV:5 * DCONV] = kc[g * 128:(g + 1) * 128]
        cw[:, 5 * DCONV:6 * DCONV] = vc[g * 128:(g + 1) * 128]

        # phase-C residual x: feature-major, cols = [b0 tokens | b1 tokens]
        xc = np.zeros((C, TOK), np.float32)
        for b in range(B):
            xc[:, b * 256:(b + 1) * 256] = xt[b][:, 256 * c:256 * (c + 1)]

        in_maps.append({
            "x8": x8b[beta], "xc": xc, "wq": wq, "tmq": tmq,
            "wp": wp, "tmp": tmp_s, "w1": w1, "w2": w2, "wm": wm,
            "cw": cw, "trig": trig, "msk": msk, "rotm": rotm,
        })
    return in_maps


_NC_CACHE = None


def kernel(**inputs) -> np.ndarray:
    global LAST_RESULTS, _NC_CACHE
    if _NC_CACHE is None:
        _NC_CACHE = build_nc()
    nc = _NC_CACHE
    in_maps = _host_inputs(inputs)
    res = run_bass_kernel_spmd(nc, in_maps, list(range(NCORES)), trace=TRACE)
    LAST_RESULTS = res
    out = np.zeros((B, T, C), np.float32)
    for c in range(NCORES):
        oc = res.results[c]["out"]                   # (C, TOK) feature-major
        for b in range(B):
            out[b, 256 * c:256 * (c + 1), :] = oc[:, b * 256:(b + 1) * 256].T
    return out


# revision 11
# speedup vs baseline: 1.2281x; 1.0460x over previous
"""Distributed Trainium2 kernel for the dense transformer block.

Strategy (8 NeuronCores, SPMD), v3 — (kv-group x batch)-parallel attention:
  Core c handles query group g = c//2 of batch beta = c%2 for ALL 2048
  tokens. qkv projection (fp8 DoubleRow matmuls), depthwise causal conv,
  SiLU and RoPE are fully local (no halo, no collective). Causal
  attention for the core's 4 heads is fully local: scores in bf16, exp
  written straight to fp8, AV as fp8 DoubleRow, rowsum as narrow (M=4)
  DoubleRow matmuls. Two fp8 AllToAlls (heads 01 / 23) reshard y to
  token-parallel (256 tokens of each batch per core); the proj
  accumulation is split even/odd so the second A2A hides behind the
  first half of proj.
  Phase C: proj fp8 DR + residual -> rmsnorm2 -> gated MLP in bf16
  (fp8 fails the precision budget there) -> residual.

All fp8 operands use a fixed scale of 32 (values < 6 in magnitude;
fp8e4 clips at 240). Weights are quantized per output channel on the
host; dequant scales fold into the PSUM->SBUF copies.
"""
import os
import sys

sys.path.insert(0, "/opt/trn_rl_repo")

import numpy as np
import ml_dtypes

import concourse.bass as bass
import concourse.mybir as mybir
from concourse import bacc, tile
from concourse.bass_utils import run_bass_kernel_spmd

B, T, C = 2, 2048, 2048
NH, NG, HS = 16, 4, 128
QPK = NH // NG
DCONV = 4
IM = 5632
EPS = 1e-5
NCORES = 8
NKC = C // 128       # 16
NMI = IM // 128      # 44
TOK = 512            # phase-C tokens per core (256 of each batch)
SCALE = 1.0 / float(np.sqrt(HS))
QS = 32.0            # fp8 activation scale
LN_QS = float(np.log(QS))

F32 = mybir.dt.float32
BF16 = mybir.dt.bfloat16
FP8 = mybir.dt.float8e4
AF = mybir.ActivationFunctionType
ALU = mybir.AluOpType
PM = mybir.MatmulPerfMode

DEBUG = bool(int(os.environ.get("KERNEL_DEBUG", "0")))
TRACE = bool(int(os.environ.get("KERNEL_TRACE", "0")))

LAST_RESULTS = None  # test.py reads exec_time from here


# --------------------------------------------------------------------------
# builder
# --------------------------------------------------------------------------

def build_nc():
    nc = bacc.Bacc("TRN2", target_bir_lowering=False, debug=False,
                   enable_asserts=True, num_devices=NCORES)

    # per-core inputs
    x8_d = nc.dram_tensor("x8", [128, NKC, T], FP8, kind="ExternalInput")
    xc_d = nc.dram_tensor("xc", [C, TOK], F32, kind="ExternalInput")
    wq_d = nc.dram_tensor("wq", [6, 128, NKC, 128], FP8, kind="ExternalInput")
    tmq_d = nc.dram_tensor("tmq", [128, 6], F32, kind="ExternalInput")
    wp_d = nc.dram_tensor("wp", [16, 128, NKC, 128], FP8, kind="ExternalInput")
    tmp_d = nc.dram_tensor("tmp", [128, 16], F32, kind="ExternalInput")
    w1_d = nc.dram_tensor("w1", [NMI, 128, C], BF16, kind="ExternalInput")
    w2_d = nc.dram_tensor("w2", [NMI, 128, C], BF16, kind="ExternalInput")
    wm_d = nc.dram_tensor("wm", [16, 128, IM], BF16, kind="ExternalInput")
    cw_d = nc.dram_tensor("cw", [128, 6 * DCONV], F32, kind="ExternalInput")
    trig_d = nc.dram_tensor("trig", [128, 2 * T], BF16, kind="ExternalInput")
    msk_d = nc.dram_tensor("msk", [128, 2048], BF16, kind="ExternalInput")
    rotm_d = nc.dram_tensor("rotm", [128, 128], BF16, kind="ExternalInput")
    out_d = nc.dram_tensor("out", [C, TOK], F32, kind="ExternalOutput")

    # collective buffers: chunk j goes to / comes from core j.
    # dim1 rows: head pair (0,1) for a=0, (2,3) for a=1.
    t2i = [nc.dram_tensor(f"t2i{a}", [NCORES, 256, 256], FP8, kind="Internal")
           for a in range(2)]
    t2o = [nc.dram_tensor(f"t2o{a}", [NCORES, 256, 256], FP8, kind="Internal")
           for a in range(2)]

    dbg = {}
    if DEBUG:
        dbg["sl"] = nc.dram_tensor("d_sl", [6 * 128, T], BF16, kind="ExternalOutput")
        dbg["y8"] = nc.dram_tensor("d_y8", [QPK * 128, T], BF16, kind="ExternalOutput")
        dbg["x2"] = nc.dram_tensor("d_x2", [C, TOK], F32, kind="ExternalOutput")
        dbg["rinv"] = nc.dram_tensor("d_rinv", [1, T], F32, kind="ExternalOutput")

    with tile.TileContext(nc) as tc:
        with tc.tile_pool(name="pers", bufs=1) as pers:
            # ---- constants ----
            cw_sb = pers.tile([128, 6 * DCONV], F32, tag="cw", name="cw")
            tmq_sb = pers.tile([128, 6], F32, tag="tmq", name="tmq")
            tmp_sb = pers.tile([128, 16], F32, tag="tmp", name="tmp")
            nc.sync.dma_start(cw_sb[:], cw_d[:])
            nc.sync.dma_start(tmq_sb[:], tmq_d[:])
            nc.sync.dma_start(tmp_sb[:], tmp_d[:])

            ones128 = pers.tile([128, 128], BF16, tag="ones128", name="ones128")
            ones8 = pers.tile([128, 2, 32], FP8, tag="ones8", name="ones8")
            eps1 = pers.tile([1, 1], F32, tag="eps1", name="eps1")
            lnq = pers.tile([128, 1], F32, tag="lnq", name="lnq")
            nc.gpsimd.memset(ones128[:], 1.0)
            nc.gpsimd.memset(ones8[:], 1.0)
            nc.gpsimd.memset(eps1[:], EPS)
            nc.gpsimd.memset(lnq[:], LN_QS)

            # pool spanning phases A+B (closed before C to free SBUF)
            pab_cm = tc.tile_pool(name="pab_sb", bufs=1)
            pab = pab_cm.__enter__()
            msk_sb = pab.tile([128, 2048], BF16, tag="msk", name="msk")
            nc.sync.dma_start(msk_sb[:], msk_d[:])
            qall = [pab.tile([128, T], BF16, tag=f"q{h}", name=f"q{h}")
                    for h in range(QPK)]
            kall = pab.tile([128, T], BF16, tag="kall", name="kall")
            v8t = pab.tile([128, NKC, 128], FP8, tag="v8t", name="v8t")
            y8 = [pab.tile([128, T], FP8, tag=f"y8_{h}", name=f"y8_{h}")
                  for h in range(QPK)]

            # ========================================================
            # Phases A+B interleaved at head granularity
            # ========================================================
            with tc.tile_pool(name="pa_sb", bufs=1) as pa, \
                 tc.tile_pool(name="pab_ps", bufs=1, space="PSUM") as pap, \
                 tc.tile_pool(name="pb_sb", bufs=1) as pb:
                pbp = pap
                trig_sb = pa.tile([128, 2 * T], BF16, tag="trig", name="trig")
                nc.sync.dma_start(trig_sb[:], trig_d[:])
                x8 = pa.tile([128, NKC, T], FP8, tag="x8", name="x8")
                for qa in range(4):
                    nc.sync.dma_start(x8[:, qa * 4:(qa + 1) * 4, :],
                                      x8_d[:, qa * 4:(qa + 1) * 4, :])
                wq_sb = [pa.tile([128, NKC, 128], FP8, tag=f"wq{m}", name=f"wq{m}")
                         for m in range(6)]
                for m in range(6):
                    nc.sync.dma_start(wq_sb[m][:], wq_d[m])

                # ---- rmsnorm scale (tq-major; squares split Act/DVE) ----
                rinvb = pa.tile([128, T], F32, tag="rinvb", name="rinvb")
                for tq in range(4):
                    sl512 = slice(tq * 512, (tq + 1) * 512)
                    ss_ps = pap.tile([128, 512], F32, tag="qk", bufs=2, name="ss")
                    for kk in range(NKC):
                        xsq = pa.tile([128, 512], BF16, tag="xsq", bufs=4, name="xsq")
                        if kk % 2 == 0:
                            nc.scalar.activation(xsq[:], x8[:, kk, sl512], AF.Square)
                        else:
                            nc.vector.tensor_mul(xsq[:], x8[:, kk, sl512],
                                                 x8[:, kk, sl512])
                        nc.tensor.matmul(ss_ps[:], ones128[:], xsq[:],
                                         start=(kk == 0), stop=(kk == NKC - 1))
                    rt = pa.tile([1, 512], F32, tag="rt", bufs=2, name="rt")
                    nc.scalar.activation(rt[:], ss_ps[0:1, :], AF.Sqrt,
                                         bias=eps1[:], scale=1.0 / (C * QS * QS))
                    rinv = pa.tile([1, 512], F32, tag="rinv", bufs=2, name="rinv")
                    nc.vector.reciprocal(rinv[:], rt[:])
                    nc.gpsimd.partition_broadcast(rinvb[:, sl512], rinv[:])
                if DEBUG:
                    nc.sync.dma_start(dbg["rinv"][:], rinvb[0:1, :])

                pre = {}

                def qkv_mtile(m):
                    """qkv DR matmuls + dequant for local m-tile."""
                    pre[m] = pa.tile([128, DCONV - 1 + T], BF16,
                                     tag=f"pre{m}", name=f"pre{m}")
                    nc.gpsimd.memset(pre[m][:, 0:DCONV - 1], 0.0)
                    for tq in range(4):
                        sl512 = slice(tq * 512, (tq + 1) * 512)
                        qk_ps = pap.tile([128, 512], F32, tag="qk", bufs=2,
                                         name="qk")
                        for jp in range(NKC // 2):
                            nc.tensor.matmul(
                                qk_ps[:], wq_sb[m][:, 2 * jp:2 * jp + 2, :],
                                x8[:, 2 * jp:2 * jp + 2, sl512],
                                start=(jp == 0), stop=(jp == NKC // 2 - 1),
                                perf_mode=PM.DoubleRow)
                        nc.vector.scalar_tensor_tensor(
                            pre[m][:, DCONV - 1 + tq * 512:DCONV - 1 + (tq + 1) * 512],
                            qk_ps[:], tmq_sb[:, m:m + 1], rinvb[:, sl512],
                            op0=ALU.mult, op1=ALU.mult)

                def convrope_mtile(m):
                    """causal conv + silu (+rope for q/k, fp8 transpose for v)."""
                    for tq in range(4):
                        a0 = tq * 512
                        acc = pa.tile([128, 512], F32, tag="cacc", bufs=3,
                                      name="cacc")
                        nc.scalar.activation(acc[:], pre[m][:, a0:a0 + 512],
                                             AF.Copy, scale=cw_sb[:, m * 4:m * 4 + 1])
                        for j in range(1, DCONV):
                            nc.vector.scalar_tensor_tensor(
                                acc[:], pre[m][:, a0 + j:a0 + j + 512],
                                cw_sb[:, m * 4 + j:m * 4 + j + 1], acc[:],
                                op0=ALU.mult, op1=ALU.add)
                        if m < 5:
                            sl = pa.tile([128, 512], BF16, tag="sl", bufs=3,
                                         name="sl")
                            nc.scalar.activation(sl[:], acc[:], AF.Silu)
                            if DEBUG:
                                nc.sync.dma_start(
                                    dbg["sl"][m * 128:(m + 1) * 128, a0:a0 + 512],
                                    sl[:])
                            dst = qall[m][:] if m < 4 else kall[:]
                            # rot = [x2; x1] via partition-swap DMAs; the sign
                            # for the rotate-half lives in the sin table
                            rot_sb = pa.tile([128, 512], BF16, tag="rsb", bufs=3,
                                             name="rsb")
                            nc.sync.dma_start(rot_sb[0:64, :], sl[64:128, :])
                            nc.sync.dma_start(rot_sb[64:128, :], sl[0:64, :])
                            tt1 = pa.tile([128, 512], BF16, tag="tt1", bufs=3,
                                          name="tt1")
                            nc.vector.tensor_mul(tt1[:], sl[:],
                                                 trig_sb[:, a0:a0 + 512])
                            tt2 = pa.tile([128, 512], BF16, tag="tt2", bufs=3,
                                          name="tt2")
                            nc.vector.tensor_mul(tt2[:], rot_sb[:],
                                                 trig_sb[:, T + a0:T + a0 + 512])
                            nc.vector.tensor_add(dst[:, a0:a0 + 512], tt1[:], tt2[:])
                        else:
                            vsl = pa.tile([128, 512], BF16, tag="vsl", bufs=2,
                                          name="vsl")
                            nc.scalar.activation(vsl[:], acc[:], AF.Silu)
                            if DEBUG:
                                nc.sync.dma_start(
                                    dbg["sl"][5 * 128:6 * 128, a0:a0 + 512], vsl[:])
                            vt = pa.tile([128, 4, 128], BF16, tag="vt", bufs=2,
                                         name="vt")
                            for i in range(4):
                                nc.sync.dma_start_transpose(
                                    vt[:, i, :], vsl[:, i * 128:(i + 1) * 128])
                            with nc.allow_low_precision(reason="fp8 v"):
                                nc.scalar.activation(
                                    v8t[:, tq * 4:(tq + 1) * 4, :], vt[:],
                                    AF.Copy, scale=QS)

                def attn_head(h):
                    """causal attention for local head h -> y8[h]."""
                    for bp in range(4):
                        npair = 2 * (bp + 1)
                        # diagonal pairs first so the AV tail never waits on
                        # the mask multiply
                        order = [npair - 2, npair - 1] + list(range(npair - 2))
                        o_ps = pbp.tile([128, 512], F32, tag="o", bufs=1, name="o")
                        rs_ps = pbp.tile([32, 512], F32, tag="rs", bufs=1, name="rs")
                        p8s = {}
                        nav = [0]

                        def av_pair(jp):
                            nc.tensor.matmul(
                                o_ps[:], v8t[:, 2 * jp:2 * jp + 2, :], p8s[jp][:],
                                start=(nav[0] == 0), stop=(nav[0] == npair - 1),
                                perf_mode=PM.DoubleRow)
                            nc.tensor.matmul(
                                rs_ps[:], ones8[:], p8s[jp][:],
                                start=(nav[0] == 0), stop=(nav[0] == npair - 1),
                                perf_mode=PM.DoubleRow)
                            nav[0] += 1

                        for idx, jp in enumerate(order):
                            s_ps = pbp.tile([128, 2, 512], F32, tag="s", bufs=2,
                                            name="s")
                            for i in range(2):
                                nc.tensor.matmul(
                                    s_ps[:, i, :],
                                    kall[:, (jp * 2 + i) * 128:(jp * 2 + i + 1) * 128],
                                    qall[h][:, bp * 512:(bp + 1) * 512],
                                    start=True, stop=True)
                            p8 = pb.tile([128, 2, 512], FP8, tag="p8", bufs=5,
                                         name="p8")
                            p8s[jp] = p8
                            with nc.allow_low_precision(reason="fp8 probs"):
                                if idx < 2:  # diagonal pair: mask
                                    pd = pb.tile([128, 2, 512], BF16, tag="pd",
                                                 bufs=2, name="pd")
                                    nc.scalar.activation(pd[:], s_ps[:], AF.Exp,
                                                         bias=lnq[:], scale=SCALE)
                                    mof = 0 if idx == 0 else 1024
                                    nc.vector.tensor_mul(
                                        p8[:].rearrange("p a b -> p (a b)"),
                                        pd[:].rearrange("p a b -> p (a b)"),
                                        msk_sb[:, mof:mof + 1024])
                                else:
                                    nc.scalar.activation(p8[:], s_ps[:], AF.Exp,
                                                         bias=lnq[:], scale=SCALE)
                            if idx >= 2:
                                av_pair(order[idx - 2])
                        av_pair(order[npair - 2])
                        av_pair(order[npair - 1])
                        rho = pb.tile([1, 512], F32, tag="rho", bufs=2, name="rho")
                        nc.vector.reciprocal(rho[:], rs_ps[0:1, :])
                        rhob = pb.tile([128, 512], F32, tag="rhob", bufs=2,
                                       name="rhob")
                        nc.gpsimd.partition_broadcast(rhob[:], rho[:])
                        with nc.allow_low_precision(reason="fp8 y"):
                            nc.vector.tensor_mul(
                                y8[h][:, bp * 512:(bp + 1) * 512], o_ps[:], rhob[:])
                    if DEBUG:
                        y8b = pb.tile([128, T], BF16, tag="y8b", bufs=1, name="y8b")
                        nc.scalar.copy(y8b[:], y8[h][:])
                        nc.sync.dma_start(dbg["y8"][h * 128:(h + 1) * 128, :],
                                          y8b[:])

                def fire_a2a(a):
                    """AllToAll for head pair (2a, 2a+1)."""
                    for hh in range(2):
                        for j in range(NCORES):
                            nc.sync.dma_start(
                                t2i[a][j, hh * 128:(hh + 1) * 128, :],
                                y8[2 * a + hh][:, 256 * j:256 * (j + 1)])
                    nc.gpsimd.collective_compute(
                        "AllToAll", ALU.bypass,
                        replica_groups=[list(range(NCORES))],
                        ins=[t2i[a][:].opt()], outs=[t2o[a][:].opt()])

                # ---- emission schedule: 1-mtile software pipeline so
                # PE (qkv m) overlaps DVE/Act (conv/rope m-1, attention) ----
                qkv_mtile(4)            # k
                qkv_mtile(5)            # v
                convrope_mtile(4)
                qkv_mtile(0)
                convrope_mtile(5)
                qkv_mtile(1)
                convrope_mtile(0)
                attn_head(0)
                qkv_mtile(2)
                convrope_mtile(1)
                attn_head(1)
                fire_a2a(0)
                qkv_mtile(3)
                convrope_mtile(2)
                attn_head(2)
                convrope_mtile(3)
                attn_head(3)
                fire_a2a(1)

            pab_cm.__exit__(None, None, None)

            # ============================================================
            # Phase C: proj (fp8 DR, even/odd split) + residual, norm2,
            #          MLP (bf16), output
            # ============================================================
            with tc.tile_pool(name="pc_sb", bufs=1) as pc_, \
                 tc.tile_pool(name="pc_ps", bufs=1, space="PSUM") as pcp:
                x2 = [pc_.tile([128, TOK], F32, tag=f"x2_{i}", name=f"x2_{i}")
                      for i in range(NKC)]
                n2 = [pc_.tile([128, TOK], BF16, tag=f"n2_{i}", name=f"n2_{i}")
                      for i in range(NKC)]
                h_t = [pc_.tile([128, TOK], BF16, tag=f"h{i}", name=f"h{i}")
                       for i in range(NMI)]

                with tc.tile_pool(name="pc0", bufs=1) as pc0:
                    xc = [pc0.tile([128, TOK], F32, tag=f"xc{i}", name=f"xc{i}")
                          for i in range(NKC)]
                    for kk in range(NKC):
                        nc.sync.dma_start(xc[kk][:], xc_d[kk * 128:(kk + 1) * 128, :])
                    # gather y8: ysb[p, kk, tok], kk = g'*4 + h
                    ysb = pc0.tile([128, NKC, TOK], FP8, tag="ysb", name="ysb")
                    for a in range(2):
                        for hh in range(2):
                            for gp in range(4):
                                for b in range(2):
                                    nc.sync.dma_start(
                                        ysb[:, gp * 4 + 2 * a + hh,
                                            b * 256:(b + 1) * 256],
                                        t2o[a][2 * gp + b,
                                               hh * 128:(hh + 1) * 128, :])
                    wp_sb = [pc0.tile([128, NKC, 128], FP8, tag=f"wp{mo}",
                                      name=f"wp{mo}") for mo in range(16)]
                    for mo in range(16):
                        nc.sync.dma_start(wp_sb[mo][:], wp_d[mo])
                    # proj: per block, accumulate head-01 pairs (jp even,
                    # first A2A) then head-23 pairs (jp odd, second A2A)
                    evens = [0, 2, 4, 6]
                    odds = [1, 3, 5, 7]
                    for blk in (range(0, 6), range(6, 11), range(11, 16)):
                        mm_tiles = {}
                        for mo in blk:
                            mm_ps = pcp.tile([128, TOK], F32, tag="mm", bufs=6,
                                             name="mm")
                            mm_tiles[mo] = mm_ps
                            for ij, jp in enumerate(evens):
                                nc.tensor.matmul(
                                    mm_ps[:], wp_sb[mo][:, 2 * jp:2 * jp + 2, :],
                                    ysb[:, 2 * jp:2 * jp + 2, :],
                                    start=(ij == 0), stop=False,
                                    perf_mode=PM.DoubleRow)
                        for mo in blk:
                            for ij, jp in enumerate(odds):
                                nc.tensor.matmul(
                                    mm_tiles[mo][:],
                                    wp_sb[mo][:, 2 * jp:2 * jp + 2, :],
                                    ysb[:, 2 * jp:2 * jp + 2, :],
                                    start=False, stop=(ij == len(odds) - 1),
                                    perf_mode=PM.DoubleRow)
                            nc.vector.scalar_tensor_tensor(
                                x2[mo][:], mm_tiles[mo][:], tmp_sb[:, mo:mo + 1],
                                xc[mo][:], op0=ALU.mult, op1=ALU.add)
                            if DEBUG:
                                nc.sync.dma_start(
                                    dbg["x2"][mo * 128:(mo + 1) * 128, :], x2[mo][:])

                ss2 = pcp.tile([128, TOK], F32, tag="nrm", bufs=2, name="nrm")
                for kk in range(NKC):
                    x2sq = pc_.tile([128, TOK], BF16, tag="x2sq", bufs=3, name="x2sq")
                    nc.scalar.activation(x2sq[:], x2[kk][:], AF.Square)
                    nc.tensor.matmul(ss2[:], ones128[:], x2sq[:],
                                     start=(kk == 0), stop=(kk == NKC - 1))
                rt2 = pc_.tile([1, TOK], F32, tag="rt2", bufs=1, name="rt2")
                nc.scalar.activation(rt2[:], ss2[0:1, :], AF.Sqrt, bias=eps1[:],
                                     scale=1.0 / C)
                rinv2 = pc_.tile([1, TOK], F32, tag="rinv2", bufs=1, name="rinv2")
                nc.vector.reciprocal(rinv2[:], rt2[:])
                rb2 = pc_.tile([128, TOK], F32, tag="rb2", bufs=1, name="rb2")
                nc.gpsimd.partition_broadcast(rb2[:], rinv2[:])
                for kk in range(NKC):
                    nc.vector.tensor_mul(n2[kk][:], x2[kk][:], rb2[:])

                for mi in range(NMI):
                    w1_sb = pc_.tile([128, C], BF16, tag="wst", bufs=3, name="wst")
                    nc.sync.dma_start(w1_sb[:], w1_d[mi])
                    h1_ps = pcp.tile([128, TOK], F32, tag="mm", bufs=6, name="mm")
                    for kk in range(NKC):
                        nc.tensor.matmul(h1_ps[:],
                                         w1_sb[:, kk * 128:(kk + 1) * 128],
                                         n2[kk][:],
                                         start=(kk == 0), stop=(kk == NKC - 1))
                    s1 = pc_.tile([128, TOK], BF16, tag="s1", bufs=2, name="s1")
                    nc.scalar.activation(s1[:], h1_ps[:], AF.Silu)
                    w2_sb = pc_.tile([128, C], BF16, tag="wst", bufs=3, name="wst")
                    nc.sync.dma_start(w2_sb[:], w2_d[mi])
                    h2_ps = pcp.tile([128, TOK], F32, tag="mm", bufs=6, name="mm")
                    for kk in range(NKC):
                        nc.tensor.matmul(h2_ps[:],
                                         w2_sb[:, kk * 128:(kk + 1) * 128],
                                         n2[kk][:],
                                         start=(kk == 0), stop=(kk == NKC - 1))
                    nc.vector.tensor_mul(h_t[mi][:], s1[:], h2_ps[:])

                with tc.tile_pool(name="pcm", bufs=1) as pcm:
                    for mo in range(16):
                        wm_sb = pcm.tile([128, IM], BF16, tag="wm", bufs=2, name="wm")
                        nc.sync.dma_start(wm_sb[:], wm_d[mo])
                        mp_ps = pcp.tile([128, TOK], F32, tag="mm", bufs=6, name="mm")
                        for ki in range(NMI):
                            nc.tensor.matmul(mp_ps[:],
                                             wm_sb[:, ki * 128:(ki + 1) * 128],
                                             h_t[ki][:],
                                             start=(ki == 0), stop=(ki == NMI - 1))
                        outsb = pc_.tile([128, TOK], F32, tag="outsb", bufs=2,
                                         name="outsb")
                        nc.vector.tensor_add(outsb[:], x2[mo][:], mp_ps[:])
                        nc.sync.dma_start(out_d[mo * 128:(mo + 1) * 128, :], outsb[:])

    nc.compile()
    return nc


# --------------------------------------------------------------------------
# host-side prep / gather
# --------------------------------------------------------------------------

def _q8(a):
    return np.clip(a, -240.0, 240.0).astype(ml_dtypes.float8_e4m3)


def _prep_fp8_lhsT(w, nm, nk):
    """w: (out, in) f32 -> (lhsT fp8 [nm,128,nk,128], scales f32 [128,nm])
    with per-output-channel absmax quantization. Dequant scale includes
    the 1/QS for the fp8 rhs activations."""
    o, i = w.shape
    assert o == nm * 128 and i == nk * 128
    r = w.reshape(nm, 128, nk, 128).transpose(0, 3, 2, 1)  # (m, p, k, c)
    amax = np.abs(r).max(axis=(1, 2))                      # (m, c)
    amax = np.maximum(amax, 1e-30)
    q = _q8(r * (240.0 / amax[:, None, None, :]))
    scales = np.ascontiguousarray((amax / (240.0 * QS)).T).astype(np.float32)
    return np.ascontiguousarray(q), scales


def _prep_lhsT(w, nm, nk):
    """w: (out, in) f32 -> (nm, 128, nk*128) bf16 where
    prep[m][p][k*128+c] = w[m*128+c, k*128+p]."""
    o, i = w.shape
    assert o == nm * 128 and i == nk * 128
    r = w.reshape(nm, 128, nk, 128).transpose(0, 3, 2, 1)
    return np.ascontiguousarray(r.reshape(nm, 128, nk * 128)).astype(ml_dtypes.bfloat16)


def _host_inputs(inputs):
    x = np.asarray(inputs["x"], np.float32)          # (B, T, C)
    cos = np.asarray(inputs["cos"], np.float32)      # (T, 64)
    sin = np.asarray(inputs["sin"], np.float32)
    n1w = np.asarray(inputs["norm1_w"], np.float32)
    n2w = np.asarray(inputs["norm2_w"], np.float32)

    attn_w = np.asarray(inputs["attn_w"], np.float32) * n1w[None, :]
    fc1_w = np.asarray(inputs["fc1_w"], np.float32) * n2w[None, :]
    fc2_w = np.asarray(inputs["fc2_w"], np.float32) * n2w[None, :]
    proj_w = np.asarray(inputs["proj_w"], np.float32)
    mlp_w = np.asarray(inputs["mlp_proj_w"], np.float32)

    wq_all, tmq_all = _prep_fp8_lhsT(attn_w, NH + 2 * NG, NKC)  # (24,128,16,128)
    wp, tmp_s = _prep_fp8_lhsT(proj_w, 16, NKC)
    w1 = _prep_lhsT(fc1_w, NMI, NKC)
    w2 = _prep_lhsT(fc2_w, NMI, NKC)
    wm = _prep_lhsT(mlp_w, 16, NMI)

    qc = np.asarray(inputs["qconv_w"], np.float32)
    kc = np.asarray(inputs["kconv_w"], np.float32)
    vc = np.asarray(inputs["vconv_w"], np.float32)

    # masks: mskA for pair npair-2 (tk rel = i*128+p), mskB for npair-1
    p = np.arange(128)[:, None]
    f = np.arange(512)[None, :]
    mskA = np.concatenate([(p <= f), (p + 128 <= f)], axis=1)
    mskB = np.concatenate([(p + 256 <= f), (p + 384 <= f)], axis=1)
    msk = np.concatenate([mskA, mskB], axis=1).astype(np.float32)
    msk = msk.astype(ml_dtypes.bfloat16)

    rotm = np.zeros((128, 128), np.float32)
    for m in range(64):
        rotm[m + 64, m] = -1.0
        rotm[m, m + 64] = 1.0
    rotm = rotm.astype(ml_dtypes.bfloat16)

    # trig [128, 2T]: cols 0:T cos (64-halves stacked), T:2T sin
    cosT = cos.T                                     # (64, T)
    sinT = sin.T
    cs = np.concatenate([cosT, cosT], axis=0)        # (128, T)
    ss = np.concatenate([-sinT, sinT], axis=0)       # rotate-half sign folded
    trig = np.ascontiguousarray(
        np.concatenate([cs, ss], axis=1)).astype(ml_dtypes.bfloat16)

    # x8 per batch: [128, NKC, T] with x8[p,kk,t] = q8(32*x[beta,t,kk*128+p])
    xt = x.transpose(0, 2, 1)                        # (B, C, T)
    x8b = []
    for beta in range(B):
        a = xt[beta].reshape(NKC, 128, T).transpose(1, 0, 2)  # (128, NKC, T)
        x8b.append(np.ascontiguousarray(_q8(a * QS)))

    in_maps = []
    for c in range(NCORES):
        g, beta = c // 2, c % 2
        msel = [g * 6 + s for s in range(6)]
        wq = np.ascontiguousarray(wq_all[msel])
        tmq = np.ascontiguousarray(tmq_all[:, msel])
        cw = np.zeros((128, 6 * DCONV), np.float32)
        for s in range(QPK):
            cw[:, s * DCONV:(s + 1) * DCONV] = qc[(g * QPK + s) * 128:(g * QPK + s + 1) * 128]
        cw[:, 4 * DCONV:5 * DCONV] = kc[g * 128:(g + 1) * 128]
        cw[:, 5 * DCONV:6 * DCONV] = vc[g * 128:(g + 1) * 128]

        # phase-C residual x: feature-major, cols = [b0 tokens | b1 tokens]
        xc = np.zeros((C, TOK), np.float32)
        for b in range(B):
            xc[:, b * 256:(b + 1) * 256] = xt[b][:, 256 * c:256 * (c + 1)]

        in_maps.append({
            "x8": x8b[beta], "xc": xc, "wq": wq, "tmq": tmq,
            "wp": wp, "tmp": tmp_s, "w1": w1, "w2": w2, "wm": wm,
            "cw": cw, "trig": trig, "msk": msk, "rotm": rotm,
        })
    return in_maps


_NC_CACHE = None


def kernel(**inputs) -> np.ndarray:
    global LAST_RESULTS, _NC_CACHE
    if _NC_CACHE is None:
        _NC_CACHE = build_nc()
    nc = _NC_CACHE
    in_maps = _host_inputs(inputs)
    res = run_bass_kernel_spmd(nc, in_maps, list(range(NCORES)), trace=TRACE)
    LAST_RESULTS = res
    out = np.zeros((B, T, C), np.float32)
    for c in range(NCORES):
        oc = res.results[c]["out"]                   # (C, TOK) feature-major
        for b in range(B):
            out[b, 256 * c:256 * (c + 1), :] = oc[:, b * 256:(b + 1) * 256].T
    return out
